# revision 6
# baseline (speedup 1.0000x reference)
"""Trainium2 Bass kernel for nn_LocalDotAttention (B=32, TX=4096, D=1024, WINDOW=4).

Strategy: the context tensor (512 MiB) is only read at 8 data-dependent rows per
batch element, so the kernel computes the predictive position on-device and
gathers just those 256 rows with an indirect DMA.  The attention math (which
needs the full W_p1/W_in weights on every core that computes it) is replicated
across all 8 cores at B=32, while the large output projection W_out is
column-sharded 8 ways — each core produces a disjoint 128-column slice of
h_tilde.  This puts ~8 MiB of DRAM traffic on each core instead of the ~14 MiB
a pure batch shard would need.

kernel(**inputs) takes the full unsharded inputs and returns (h_tilde, attn).
"""

import os

os.environ.setdefault("JAX_PLATFORMS", "")

from contextlib import ExitStack

import numpy as np

import concourse.bass as bass
import concourse.tile as tile
from concourse import bacc, mybir
from concourse.bass_utils import run_bass_kernel_spmd
from concourse.masks import make_identity

dt = mybir.dt
AF = mybir.ActivationFunctionType
ALU = mybir.AluOpType
AX = mybir.AxisListType

B, TX, D = 32, 4096, 1024
WINDOW = 4
W2 = 2 * WINDOW          # 8 gathered rows per batch element
NB = B * W2              # 256 gathered rows total
N_CORES = 8
HSH = D // N_CORES       # 128 h_tilde columns per core
P = 128
KC = D // P              # 8 contraction chunks of 128 over D
F1 = D // 2              # 512 (W_p1 output dim)


def _build_program():
    nc = bacc.Bacc("TRN2", target_bir_lowering=False, debug=False,
                   num_devices=N_CORES)

    in_t = nc.dram_tensor("in_t", [D, B], dt.float32, kind="ExternalInput")
    w_p1t = nc.dram_tensor("w_p1t", [D, F1], dt.float32, kind="ExternalInput")
    w_p2t = nc.dram_tensor("w_p2t", [F1, 1], dt.float32, kind="ExternalInput")
    w_int = nc.dram_tensor("w_int", [D, D], dt.float32, kind="ExternalInput")
    w_ot = nc.dram_tensor("w_ot", [2 * D, HSH], dt.float32, kind="ExternalInput")
    ctx = nc.dram_tensor("ctx", [B * TX, D], dt.float32, kind="ExternalInput")
    s_sel = nc.dram_tensor("s_sel", [B, NB], dt.float32, kind="ExternalInput")
    e_sel = nc.dram_tensor("e_sel", [NB, B], dt.float32, kind="ExternalInput")
    w_off = nc.dram_tensor("w_off", [P, 1], dt.float32, kind="ExternalInput")
    b_off = nc.dram_tensor("b_off", [P, 2], dt.float32, kind="ExternalInput")
    h_out = nc.dram_tensor("h_out", [B, HSH], dt.float32, kind="ExternalOutput")
    attn_out = nc.dram_tensor("attn_out", [1, NB], dt.float32, kind="ExternalOutput")

    with tile.TileContext(nc) as tc, ExitStack() as ectx:
        cpool = ectx.enter_context(tc.tile_pool(name="consts", bufs=1))
        wpool = ectx.enter_context(tc.tile_pool(name="weights", bufs=1))
        spool = ectx.enter_context(tc.tile_pool(name="small", bufs=1))
        bigpool = ectx.enter_context(tc.tile_pool(name="big", bufs=2))
        ps_scr = ectx.enter_context(tc.tile_pool(name="ps_scr", bufs=2, space="PSUM"))
        ps_rep = ectx.enter_context(tc.tile_pool(name="ps_rep", bufs=2, space="PSUM"))

        ident = cpool.tile([P, P], dt.float32)
        make_identity(nc, ident[:])

        # ---- resident weight loads (column-major, contraction dim on partitions)
        sb_in = wpool.tile([P, KC * B], dt.float32)
        nc.sync.dma_start(sb_in[:].rearrange("p (k b) -> p k b", k=KC),
                          in_t.ap().rearrange("(k p) b -> p k b", p=P))
        sb_wp1 = wpool.tile([P, KC * F1], dt.float32)
        nc.sync.dma_start(sb_wp1[:].rearrange("p (k f) -> p k f", k=KC),
                          w_p1t.ap().rearrange("(k p) f -> p k f", p=P))
        sb_wp2 = wpool.tile([P, F1 // P], dt.float32)
        nc.sync.dma_start(sb_wp2[:].rearrange("p (k f) -> p k f", k=F1 // P),
                          w_p2t.ap().rearrange("(k p) f -> p k f", p=P))
        sb_win = wpool.tile([P, KC * D], dt.float32)
        nc.sync.dma_start(sb_win[:].rearrange("p (k f) -> p k f", k=KC),
                          w_int.ap().rearrange("(k p) f -> p k f", p=P))
        sb_wo = wpool.tile([P, 2 * KC * HSH], dt.float32)
        nc.sync.dma_start(sb_wo[:].rearrange("p (k f) -> p k f", k=2 * KC),
                          w_ot.ap().rearrange("(k p) f -> p k f", p=P))
        sb_s = cpool.tile([B, NB], dt.float32)
        nc.sync.dma_start(sb_s[:], s_sel.ap())
        sb_e = cpool.tile([P, 2 * B], dt.float32)
        nc.sync.dma_start(sb_e[:].rearrange("p (c b) -> p c b", c=2),
                          e_sel.ap().rearrange("(c p) b -> p c b", p=P))

        # ---- ptT = tanh(W_p1 @ input.T) as [512, 32] in 4 partition chunks
        # tanh(x) = 2/(1+exp(-2x)) - 1   (Exp is ~2 ULP; LUT tanh accuracy unknown)
        ptT = spool.tile([P, (F1 // P) * B], dt.float32)
        for m in range(F1 // P):
            ps_pt = ps_scr.tile([P, B], dt.float32, space="PSUM", tag="scratch")
            for k in range(KC):
                nc.tensor.matmul(
                    ps_pt[:],
                    lhsT=sb_wp1[:, k * F1 + m * P:k * F1 + (m + 1) * P],
                    rhs=sb_in[:, k * B:(k + 1) * B],
                    start=(k == 0), stop=(k == KC - 1))
            t_exp = spool.tile([P, B], dt.float32, tag="t_exp")
            nc.scalar.activation(out=t_exp[:], in_=ps_pt[:], func=AF.Exp, scale=-2.0)
            nc.vector.tensor_scalar_add(out=t_exp[:], in0=t_exp[:], scalar1=1.0)
            t_rec = spool.tile([P, B], dt.float32, tag="t_rec")
            nc.vector.reciprocal(t_rec[:], t_exp[:])
            nc.scalar.activation(out=ptT[:, m * B:(m + 1) * B], in_=t_rec[:],
                                 func=AF.Copy, scale=2.0, bias=-1.0)

        # ---- pt2 = pt @ W_p2.T  -> [32, 1];  x = TX*sigmoid(pt2) - WINDOW
        ps_pt2 = ps_scr.tile([B, 1], dt.float32, space="PSUM", tag="scratch")
        for m in range(F1 // P):
            nc.tensor.matmul(ps_pt2[:], lhsT=ptT[:, m * B:(m + 1) * B],
                             rhs=sb_wp2[:, m:m + 1],
                             start=(m == 0), stop=(m == F1 // P - 1))
        ex2 = spool.tile([B, 1], dt.float32)
        nc.scalar.activation(out=ex2[:], in_=ps_pt2[:], func=AF.Exp, scale=-1.0)
        nc.vector.tensor_scalar_add(out=ex2[:], in0=ex2[:], scalar1=1.0)
        rec2 = spool.tile([B, 1], dt.float32)
        nc.vector.reciprocal(rec2[:], ex2[:])
        x_sb = spool.tile([B, 1], dt.float32)
        nc.scalar.activation(out=x_sb[:], in_=rec2[:], func=AF.Copy,
                             scale=float(TX), bias=-float(WINDOW))

        # ---- bl = floor(x) exactly (x > 0 here so trunc == floor), any-rounding-safe
        i0 = spool.tile([B, 1], dt.int32)
        nc.vector.tensor_copy(i0[:], x_sb[:])
        f0 = spool.tile([B, 1], dt.float32)
        nc.vector.tensor_copy(f0[:], i0[:])
        delta = spool.tile([B, 1], dt.float32)
        nc.vector.tensor_tensor(out=delta[:], in0=f0[:], in1=x_sb[:], op=ALU.is_gt)
        fl = spool.tile([B, 1], dt.float32)
        nc.vector.tensor_tensor(out=fl[:], in0=f0[:], in1=delta[:], op=ALU.subtract)

        # ---- gather indices, canonical one-index-per-partition layout:
        # gidx_c[p] = clip(bl[16c + p//8] + p%8, 0, TX-1) + TX*(16c + p//8)
        sb_woff = cpool.tile([P, 1], dt.float32)
        nc.sync.dma_start(sb_woff[:], w_off.ap())
        sb_boff = cpool.tile([P, 2], dt.float32)
        nc.sync.dma_start(sb_boff[:], b_off.ap())
        cw = []
        for c in range(2):
            ps_blr = ps_scr.tile([P, 1], dt.float32, space="PSUM", tag="scratch")
            nc.tensor.matmul(ps_blr[:], lhsT=sb_s[:, c * P:(c + 1) * P],
                             rhs=fl[:], start=True, stop=True)
            gidx = spool.tile([P, 1], dt.float32, tag=f"gidx{c}")
            nc.vector.tensor_tensor(out=gidx[:], in0=ps_blr[:], in1=sb_woff[:],
                                    op=ALU.add)
            nc.vector.tensor_scalar(out=gidx[:], in0=gidx[:], scalar1=0.0,
                                    scalar2=float(TX - 1), op0=ALU.max, op1=ALU.min)
            nc.vector.tensor_tensor(out=gidx[:], in0=gidx[:],
                                    in1=sb_boff[:, c:c + 1], op=ALU.add)
            gidx_i = spool.tile([P, 1], dt.int32, tag=f"gidxi{c}")
            nc.vector.tensor_copy(gidx_i[:], gidx[:])
            cw_c = bigpool.tile([P, D], dt.float32, tag=f"cw{c}")
            nc.gpsimd.indirect_dma_start(
                out=cw_c[:], out_offset=None,
                in_=ctx.ap(),
                in_offset=bass.IndirectOffsetOnAxis(ap=gidx_i[:, 0:1], axis=0))
            cw.append(cw_c)

        # ---- target = input @ W_in.T  -> [32, 1024]
        ps_tg = ps_scr.tile([B, D], dt.float32, space="PSUM", tag="scratch")
        for n in range(2):
            for k in range(KC):
                nc.tensor.matmul(
                    ps_tg[:, n * 512:(n + 1) * 512],
                    lhsT=sb_in[:, k * B:(k + 1) * B],
                    rhs=sb_win[:, k * D + n * 512:k * D + n * 512 + 512],
                    start=(k == 0), stop=(k == KC - 1))
        tg_sb = spool.tile([B, D], dt.float32)
        nc.scalar.activation(out=tg_sb[:], in_=ps_tg[:], func=AF.Copy)

        # ---- replicate target rows into the (b,w) partition layout via S matmul
        reps = []
        for c in range(2):
            ps_rp = ps_rep.tile([P, D], dt.float32, space="PSUM", tag="rep")
            for n in range(2):
                nc.tensor.matmul(ps_rp[:, n * 512:(n + 1) * 512],
                                 lhsT=sb_s[:, c * P:(c + 1) * P],
                                 rhs=tg_sb[:, n * 512:(n + 1) * 512],
                                 start=True, stop=True)
            reps.append(ps_rp)

        # ---- scores[(b,w)] = <cw row, target_b>  -> two [128, 1] columns
        sc_cols = []
        for c in range(2):
            prod = bigpool.tile([P, D], dt.float32, tag="prod")
            nc.vector.tensor_tensor(out=prod[:], in0=cw[c][:], in1=reps[c][:],
                                    op=ALU.mult)
            sc_c = spool.tile([P, 1], dt.float32, tag=f"sc{c}")
            nc.vector.tensor_reduce(out=sc_c[:], in_=prod[:], axis=AX.X, op=ALU.add)
            sc_cols.append(sc_c)

        # ---- transpose scores to a [1, 256] row, softmax over each group of 8
        sr = spool.tile([1, NB], dt.float32)
        for c in range(2):
            ps_row = ps_scr.tile([1, P], dt.float32, space="PSUM", tag="scratch")
            nc.tensor.transpose(out=ps_row[:], in_=sc_cols[c][:], identity=ident[:])
            nc.vector.tensor_copy(sr[:, c * P:(c + 1) * P], ps_row[:])
        sr3 = sr[:].rearrange("p (b w) -> p b w", w=W2)
        nm1 = spool.tile([1, B], dt.float32)
        nc.vector.tensor_reduce(out=nm1[:], in_=sr3, axis=AX.X, op=ALU.max)
        srs = spool.tile([1, NB], dt.float32)
        nm1_b = bass.AP(nm1[:].tensor, nm1[:].offset, [nm1[:].ap[0], [1, B], [0, W2]])
        nc.vector.tensor_tensor(out=srs[:].rearrange("p (b w) -> p b w", w=W2),
                                in0=sr3, in1=nm1_b, op=ALU.subtract)
        e1 = spool.tile([1, NB], dt.float32)
        nc.scalar.activation(out=e1[:], in_=srs[:], func=AF.Exp)
        s1 = spool.tile([1, B], dt.float32)
        nc.vector.tensor_reduce(out=s1[:], in_=e1[:].rearrange("p (b w) -> p b w", w=W2),
                                axis=AX.X, op=ALU.add)
        r1 = spool.tile([1, B], dt.float32)
        nc.vector.reciprocal(r1[:], s1[:])
        attn_row = spool.tile([1, NB], dt.float32)
        r1_b = bass.AP(r1[:].tensor, r1[:].offset, [r1[:].ap[0], [1, B], [0, W2]])
        nc.vector.tensor_tensor(out=attn_row[:].rearrange("p (b w) -> p b w", w=W2),
                                in0=e1[:].rearrange("p (b w) -> p b w", w=W2),
                                in1=r1_b, op=ALU.mult)
        nc.sync.dma_start(attn_out.ap(), attn_row[:])

        # ---- transpose attn back to the (b,w) partition layout, mask into A
        attn_sq = spool.tile([P, NB], dt.float32)
        nc.gpsimd.memset(attn_sq[:], 0.0)
        nc.vector.tensor_copy(attn_sq[0:1, :], attn_row[:])
        A = []
        for c in range(2):
            ps_a = ps_scr.tile([P, P], dt.float32, space="PSUM", tag="scratch")
            nc.tensor.transpose(out=ps_a[:], in_=attn_sq[:, c * P:(c + 1) * P],
                                identity=ident[:])
            A_c = spool.tile([P, B], dt.float32, tag=f"A{c}")
            nc.vector.tensor_tensor(out=A_c[:], in0=ps_a[:, 0:1].to_broadcast([P, B]),
                                    in1=sb_e[:, c * B:(c + 1) * B], op=ALU.mult)
            A.append(A_c)

        # ---- weightedT[d, b] = sum_(b,w) cw[(b,w), d] * A[(b,w), b]  -> [1024, 32]
        wt_sb = spool.tile([P, KC * B], dt.float32)
        for m in range(KC):
            ps_w = ps_scr.tile([P, B], dt.float32, space="PSUM", tag="scratch")
            for c in range(2):
                nc.tensor.matmul(ps_w[:], lhsT=cw[c][:, m * P:(m + 1) * P],
                                 rhs=A[c][:], start=(c == 0), stop=(c == 1))
            nc.scalar.activation(out=wt_sb[:, m * B:(m + 1) * B], in_=ps_w[:],
                                 func=AF.Copy)

        # ---- h = tanh([weighted, input] @ W_out.T) for this core's column slice
        ps_h = ps_scr.tile([B, HSH], dt.float32, space="PSUM", tag="scratch")
        for k in range(2 * KC):
            lhsT = (wt_sb[:, k * B:(k + 1) * B] if k < KC
                    else sb_in[:, (k - KC) * B:(k - KC + 1) * B])
            nc.tensor.matmul(ps_h[:], lhsT=lhsT,
                             rhs=sb_wo[:, k * HSH:(k + 1) * HSH],
                             start=(k == 0), stop=(k == 2 * KC - 1))
        h_sb = spool.tile([B, HSH], dt.float32)
        nc.scalar.activation(out=h_sb[:], in_=ps_h[:], func=AF.Tanh)
        nc.sync.dma_start(h_out.ap(), h_sb[:])

    nc.compile()
    return nc


_NC = None


def _get_program():
    global _NC
    if _NC is None:
        _NC = _build_program()
    return _NC


def _host_consts():
    s_sel = np.zeros((B, NB), np.float32)
    e_sel = np.zeros((NB, B), np.float32)
    for b in range(B):
        s_sel[b, b * W2:(b + 1) * W2] = 1.0
        e_sel[b * W2:(b + 1) * W2, b] = 1.0
    p = np.arange(P)
    w_off = (p % W2).astype(np.float32)[:, None]
    b_off = np.stack([TX * (p // W2), TX * (16 + p // W2)], 1).astype(np.float32)
    return s_sel, e_sel, w_off, b_off


def _make_in_maps(input, context, W_in, W_out, W_p1, W_p2):
    in_t = np.ascontiguousarray(input.T)
    w_p1t = np.ascontiguousarray(W_p1.T)
    w_p2t = np.ascontiguousarray(W_p2.T)
    w_int = np.ascontiguousarray(W_in.T)
    w_ot_full = np.ascontiguousarray(W_out.T)          # [2048, 1024]
    ctx_flat = np.ascontiguousarray(context.reshape(B * TX, D))
    s_sel, e_sel, w_off, b_off = _host_consts()
    in_maps = []
    for c in range(N_CORES):
        in_maps.append({
            "in_t": in_t,
            "w_p1t": w_p1t,
            "w_p2t": w_p2t,
            "w_int": w_int,
            "w_ot": np.ascontiguousarray(w_ot_full[:, c * HSH:(c + 1) * HSH]),
            "ctx": ctx_flat,
            "s_sel": s_sel,
            "e_sel": e_sel,
            "w_off": w_off,
            "b_off": b_off,
        })
    return in_maps


def run(inputs, trace=False, **kw):
    """Run the SPMD kernel; returns (BassKernelResults, (h, attn))."""
    nc = _get_program()
    in_maps = _make_in_maps(**{k: np.asarray(v) for k, v in inputs.items()})
    res = run_bass_kernel_spmd(nc, in_maps, core_ids=list(range(N_CORES)),
                               trace=trace, **kw)
    h = np.concatenate([res.results[c]["h_out"] for c in range(N_CORES)], axis=1)
    attn = res.results[0]["attn_out"].reshape(B, W2).astype(np.float32)
    return res, (np.ascontiguousarray(h, dtype=np.float32), attn)


def kernel(**inputs):
    _, out = run(inputs, trace=False)
    return out
